# revision 57
# baseline (speedup 1.0000x reference)
"""Grouped-Query Attention kernel for 8 Trainium2 NeuronCores.

Reference model: x[1,2048,2048] -> Q(32 heads x 64) / K,V(8 kv heads x 64),
per-head RMS-norm(Q,K) + RoPE, causal softmax attention, out-projection.

Sharding (tensor-parallel over heads): core c owns Q heads 4c..4c+3 and KV
head c (exactly its GQA group) and W_out rows [256c : 256c+256).  Each core
computes a full-shape partial output; the host sums the 8 partials (the
unshard step for a row-sharded W_out).

On-core strategy (fp16 matmul path, ~3.3x over the fp32 version in the
cost model):
  - all matmul inputs are float16 (PE runs 1 cycle/row vs 4 for fp32);
    accumulation stays fp32 in PSUM.  fp16 keeps 10 mantissa bits, and
    RMS-norm bounds |q.k| <= 64 so exp(s/8) <= e^8 ~ 3e3 < fp16 max.
    Measured on HW: rel err ~6e-4 vs the fp32 reference.
  - x is loaded TRANSPOSED via the DMA XBAR (dma_start_transpose, 2-byte
    dtype, quarter-major order): no PE transposes / PSUM round-trips for x.
  - q/k/v projections run as ONE fused [128,384] PSUM accumulation; the
    rms-norm sum-of-squares runs on Pool (from an fp16 SBUF copy of q/k,
    exploiting that RoPE is norm-preserving is NOT assumed - sums are taken
    pre-rope), rope runs fp16 on DVE, and the per-tile PE transposes of
    q/k trail the projection by two tiles so the 4-engine chain never
    stalls the PE.
  - scores are built TRANSPOSED (S^T[j,i] = k_j . q_i) so that
      * PV needs no attention-matrix transpose
      * the softmax denominator comes free via an extra ones-column in V
  - causal diagonal tiles only compute the live column sub-range
  - the two heads of a GQA pair are processed together: one exp and one
    (3D) affine_select cover both heads' score tiles, halving ACT/Pool
    instruction counts
  - masked tiles' scores run first and their PVs drain last; unmasked
    tiles run a lead-3 software pipeline, so the PE never waits on exp
  - softmax normalize is fused: 1/den (DVE reciprocal from PSUM) -> fp16
    -> partition-broadcast via two 1-partition PE matmuls -> one DVE
    multiply writes normalized fp16 ctx straight from the PV accumulator
  - the out-projection is sliced per t-tile and interleaved into the NEXT
    attention block (between head pairs) so its PSUM->SBUF copies never
    head-of-line-block the DVE queue; partial outputs are stored fp16
    (host sums in fp32)
  - q/k scales and the rotate-half signs are folded into host-precomputed
    cos/sin tables
"""

import numpy as np

T = 2048
D = 2048
NUM_HEADS = 32
NUM_KV = 8
HD = 64
N_CORES = 8
H_LOC = NUM_HEADS // N_CORES  # 4 q heads per core
EPS = 1e-6

TT = T // 128   # 16 t-tiles of 128 rows
CC = D // 128   # 16 contraction chunks
IBS = T // 512  # 4 i-blocks of 512 query positions
JBS = T // 128  # 16 j-blocks of 128 key positions

KEEP = "keep"
SKIP = "skip"
AFFINE = "affine"


def _classify_mask(mask: np.ndarray):
    """Per (ib, jb) scoresT tile: how to apply the mask.

    Returns (status[IBS][JBS], patterns[n,128,512]) where patterns are
    multiplicative keep-masks in S^T (j, i) layout for irregular tiles.
    """
    keep = ~mask
    status = [[KEEP] * JBS for _ in range(IBS)]
    pat_index: dict[bytes, int] = {}
    pats: list[np.ndarray] = []
    ii, jj = np.meshgrid(np.arange(512), np.arange(128), indexing="ij")
    for ib in range(IBS):
        for jb in range(JBS):
            sub = keep[ib * 512:(ib + 1) * 512, jb * 128:(jb + 1) * 128]
            if sub.all():
                status[ib][jb] = KEEP
            elif not sub.any():
                status[ib][jb] = SKIP
            else:
                causal = (jb * 128 + jj) <= (ib * 512 + ii)
                if (sub == causal).all():
                    status[ib][jb] = AFFINE
                else:
                    key = sub.tobytes()
                    if key not in pat_index:
                        pat_index[key] = len(pats)
                        pats.append(sub.T.astype(np.float32))  # [128 j, 512 i]
                    status[ib][jb] = ("pat", pat_index[key])
    patterns = (
        np.stack(pats) if pats else np.zeros((1, 128, 512), dtype=np.float32)
    )
    return status, patterns


def _split_multiwaits(nc):
    """walrus in this container accepts only ONE sync-wait per instruction;
    hoist extra waits onto preceding same-engine NoOps (program order on the
    engine queue preserves the gating)."""
    import bass_rust
    from concourse import mybir

    n_fixed = 0
    for fn in nc.m.functions:
        for bb in fn.blocks:
            out = []
            for ins in bb.instructions:
                si = ins.sync_info
                if si is not None and si.on_wait and len(si.on_wait) > 1:
                    waits = list(si.on_wait)
                    ups = list(si.on_update) if si.on_update else []
                    for k, w in enumerate(waits[:-1]):
                        nop = mybir.InstNoOp(
                            name=f"{ins.name}-wnop{k}", ins=[], outs=[]
                        )
                        nop.engine = ins.engine
                        nop.sync_info = bass_rust.SyncInfo(
                            on_wait=[w], on_update=[]
                        )
                        out.append(nop)
                    ins.sync_info = bass_rust.SyncInfo(
                        on_wait=[waits[-1]], on_update=ups
                    )
                    n_fixed += 1
                out.append(ins)
            bb.instructions = out
    return n_fixed


def _affine_c0(ib, jb):
    """live-column start of an AFFINE (causal-diagonal) S^T tile: columns
    i_local < (jb - 4*ib)*128 are fully masked -> skip them entirely."""
    return max(0, min(3, jb - ib * (512 // 128))) * 128


_PARTS = "full"  # test-only knob: "p1" / "noout" / "full"


def _build_program(status, n_pat, reps=1):
    import concourse.bass as bass
    import concourse.mybir as mybir
    import concourse.tile as tile
    from concourse.masks import make_identity

    f32 = mybir.dt.float32
    f16 = mybir.dt.float16
    AF = mybir.ActivationFunctionType
    AX = mybir.AxisListType

    nc = bass.Bass("TRN2", num_devices=N_CORES)
    x_d = nc.declare_dram_parameter("x", [T, D], f16, isOutput=False)
    wqkv_d = nc.declare_dram_parameter(
        "wqkv", [D, (H_LOC + 2) * HD], f16, isOutput=False)
    wo_d = nc.declare_dram_parameter("wo", [H_LOC * HD, D], f16, isOutput=False)
    cosq_d = nc.declare_dram_parameter("cosq", [T, HD], f16, isOutput=False)
    sinq_d = nc.declare_dram_parameter("sinq", [T, HD], f16, isOutput=False)
    cosk_d = nc.declare_dram_parameter("cosk", [T, HD], f16, isOutput=False)
    sink_d = nc.declare_dram_parameter("sink", [T, HD], f16, isOutput=False)
    mpat_d = nc.declare_dram_parameter(
        "mpat", [n_pat, 128, 512], f16, isOutput=False
    )
    out_d = nc.declare_dram_parameter("out", [T, D], f16, isOutput=True)

    inv_sqrt_d = float(1.0 / np.sqrt(HD))

    with tile.TileContext(nc) as tc:
      for _rep in range(reps):
        with (
            tc.tile_pool(name="const", bufs=1) as const,
            tc.tile_pool(name="persist", bufs=1) as persist,
        ):
            ident = const.tile([128, 128], f16)
            make_identity(nc, ident)
            eps_t = const.tile([128, 1], f32)
            nc.vector.memset(eps_t, EPS)
            # partition-broadcast helpers: onesE spreads a [1,512] row to
            # out partitions 0:64, onesO to 64:128 (via PE matmul)
            # partition-broadcast helpers: engine ops must start at
            # partition 0/32/64/96 and stay in-window, and den rows live at
            # psc partitions 64 (even head) / 32 (odd).  Two accumulating
            # 1-partition matmuls spread row64 -> out 0:64, row32 -> 64:128.
            ones65 = const.tile([65, 128], f16, name="ones65")
            nc.vector.memset(ones65[32:33, :], 0.0)
            nc.vector.memset(ones65[64:65, :], 0.0)
            nc.vector.memset(ones65[64:65, 0:64], 1.0)
            nc.vector.memset(ones65[32:33, 64:128], 1.0)

            # persistent across phases
            qT = persist.tile([64, H_LOC, T], f16)
            kT = persist.tile([64, T], f16)
            # v with aux columns:
            #  v_aug  [128,TT,65]:  cols 0:64 = v, col 64 = 1 (even head)
            #  v_aug2 [128,TT,128]: col 32 = 1, cols 64:128 = v (odd head)
            v_aug = persist.tile([128, TT, 65], f16)
            nc.vector.memset(v_aug[:, :, 64:65], 1.0)
            v_aug2 = persist.tile([128, TT, 128], f16)
            nc.vector.memset(v_aug2[:, :, 0:64], 0.0)
            nc.vector.memset(v_aug2[:, :, 32:33], 1.0)
            # normalized fp16 ctx^T (written straight from PSUM by the
            # fused 1/den multiply), consumed by the out-projection
            ctx16 = [persist.tile([128, T], f16, name=f"ctx16{p}")
                     for p in range(2)]

            # ---- phase 1: load x^T, project q/k/v, rms-norm + rope ----
            with (
                tc.tile_pool(name="p1w", bufs=1) as p1w,
                tc.tile_pool(name="p1t", bufs=4) as p1t,
                tc.tile_pool(name="ps1a", bufs=2, space="PSUM") as ps1a,
                tc.tile_pool(name="ps1b", bufs=3, space="PSUM") as ps1b,
            ):
                MQKV = (H_LOC + 2) * HD  # 384: q heads | k | v
                wqkv_sb = p1w.tile([128, CC, MQKV], f16)
                nc.sync.dma_start(
                    out=wqkv_sb,
                    in_=wqkv_d.rearrange("(cc p) m -> p cc m", p=128)
                )
                ctabs = {}
                for nm, dd in (("cosq", cosq_d), ("sinq", sinq_d),
                               ("cosk", cosk_d), ("sink", sink_d)):
                    tab = p1w.tile([128, TT, HD], f16, name=f"tab_{nm}")
                    nc.sync.dma_start(
                        out=tab, in_=dd.rearrange("(tt p) d -> p tt d", p=128)
                    )
                    ctabs[nm] = tab

                # x transposed via the DMA XBAR:
                #   xt_all[p, cc, t] = x[t, cc*128 + p]
                # split in t-quarters, quarter-major, so the tt=0..3
                # projections can start after ~1/4 of the transpose traffic
                xt_all = p1w.tile([128, CC, T], f16)
                for th in range(4):
                    tb = slice(th * (T // 4), (th + 1) * (T // 4))
                    for cc in range(CC):
                        nc.sync.dma_start_transpose(
                            out=xt_all[:, cc, tb],
                            in_=x_d[tb, cc * 128:(cc + 1) * 128],
                        )

                def emit_proj(tt, mid=None):
                    ps = ps1b.tile([128, MQKV], f32, tag="psqkv")
                    for cc in range(CC):
                        if cc == CC // 2 and mid is not None:
                            mid()  # splice transposes mid-accumulation
                        nc.tensor.matmul(
                            ps, xt_all[:, cc, tt * 128:(tt + 1) * 128],
                            wqkv_sb[:, cc, :],
                            start=(cc == 0), stop=(cc == CC - 1))
                    return ps

                def emit_rope(tt, ps):
                    """rms-norm + rope; q/k hop PSUM->SBUF fp16 once (ACT),
                    then all elementwise work runs fp16.  Returns (qr, kr)."""
                    QK = (H_LOC + 1) * HD  # q heads + k
                    qk16 = p1t.tile([128, QK], f16, tag="qk16")
                    nc.scalar.activation(qk16, ps[:, 0:QK], AF.Copy)
                    # v copies: ACT from PSUM, then Pool SBUF->SBUF
                    nc.scalar.activation(v_aug[:, tt, 0:64],
                                         ps[:, QK:QK + HD], AF.Copy)
                    nc.gpsimd.tensor_copy(v_aug2[:, tt, 64:128],
                                          v_aug[:, tt, 0:64])

                    # sum of squares per head (Pool), 1/rms (ACT sqrt + DVE)
                    q5 = qk16.rearrange("p (h d) -> p h d", h=H_LOC + 1)
                    sq16 = p1t.tile([128, H_LOC + 1, HD], f16, tag="sq16")
                    nc.gpsimd.tensor_mul(sq16, q5, q5)
                    ssk = p1t.tile([128, H_LOC + 1, 1], f32, tag="ssk")
                    nc.vector.reduce_sum(ssk, sq16, axis=AX.X)
                    rinv = p1t.tile([128, H_LOC + 1], f32, tag="rinv")
                    nc.scalar.activation(
                        rinv, ssk.rearrange("p h o -> p (h o)"), AF.Sqrt,
                        bias=eps_t[:, 0:1], scale=1.0 / HD)
                    nc.vector.reciprocal(rinv, rinv)
                    r16 = p1t.tile([128, H_LOC + 1, 1], f16, tag="r16")
                    nc.vector.tensor_copy(
                        r16, rinv.rearrange("p (h o) -> p h o", o=1))

                    # rope(q) * rinv_q (fp16 all the way; rms-norm commutes
                    # with rope so the 1/rms multiply comes last)
                    q3 = q5[:, 0:H_LOC, :]
                    cq = ctabs["cosq"][:, tt, :].rearrange(
                        "p (o d) -> p o d", o=1)
                    sq = ctabs["sinq"][:, tt, :].rearrange(
                        "p (o d) -> p o d", o=1)
                    qr_f = p1t.tile([128, H_LOC, HD], f16, tag="qr_f")
                    nc.vector.tensor_mul(
                        qr_f, q3, cq.to_broadcast([128, H_LOC, HD]))
                    qrot = p1t.tile([128, H_LOC, HD], f16, tag="qrot")
                    nc.vector.tensor_mul(
                        qrot[:, :, 0:32], q3[:, :, 32:64],
                        sq[:, :, 0:32].to_broadcast([128, H_LOC, 32]))
                    nc.vector.tensor_mul(
                        qrot[:, :, 32:64], q3[:, :, 0:32],
                        sq[:, :, 32:64].to_broadcast([128, H_LOC, 32]))
                    nc.vector.tensor_add(qr_f, qr_f, qrot)
                    qr = p1t.tile([128, H_LOC, HD], f16, tag="qr")
                    nc.vector.tensor_mul(
                        qr, qr_f,
                        r16[:, 0:H_LOC, :].to_broadcast([128, H_LOC, HD]))

                    # rope(k) * rinv_k
                    k1 = q5[:, H_LOC, :]
                    kr_f = p1t.tile([128, HD], f16, tag="kr_f")
                    nc.vector.tensor_mul(kr_f, k1, ctabs["cosk"][:, tt, :])
                    krot = p1t.tile([128, HD], f16, tag="krot")
                    nc.vector.tensor_mul(
                        krot[:, 0:32], k1[:, 32:64],
                        ctabs["sink"][:, tt, 0:32])
                    nc.vector.tensor_mul(
                        krot[:, 32:64], k1[:, 0:32],
                        ctabs["sink"][:, tt, 32:64])
                    nc.vector.tensor_add(kr_f, kr_f, krot)
                    kr = p1t.tile([128, HD], f16, tag="kr")
                    nc.vector.tensor_mul(
                        kr, kr_f,
                        r16[:, H_LOC, :].to_broadcast([128, HD]))
                    return qr, kr

                def emit_transpose(tt, qr, kr):
                    psqt = ps1a.tile([64, 512], f16, tag="psqt")
                    for h in range(H_LOC):
                        nc.tensor.transpose(
                            psqt[:, h * 128:(h + 1) * 128], qr[:, h, :],
                            ident)
                    # one strided DVE copy: psqt [64,(4,128)] -> qT[:,h,ttb]
                    nc.vector.tensor_copy(
                        qT[:, :, tt * 128:(tt + 1) * 128],
                        psqt.rearrange("p (h t) -> p h t", h=H_LOC))
                    pskt = ps1a.tile([64, 128], f16, tag="pskt")
                    nc.tensor.transpose(pskt, kr, ident)
                    nc.vector.tensor_copy(
                        kT[:, tt * 128:(tt + 1) * 128], pskt)

                # software-pipeline: transposes(tt-1) are emitted after
                # proj(tt) so the PE never waits on the DVE rope
                # transposes run TWO t-tiles behind the projection: the
                # rms+rope chain spans ~4 engines and needs ~2 proj-tiles
                # of PE time to finish without stalling the transposes
                from collections import deque
                lag = deque()
                for tt in range(TT):
                    mid = None
                    if len(lag) >= 2:
                        mid = (lambda p=lag.popleft():
                               emit_transpose(*p))
                    ps = emit_proj(tt, mid)
                    lag.append((tt,) + emit_rope(tt, ps))
                while lag:
                    emit_transpose(*lag.popleft())

            # ---- phase 2+3: attention + out-projection ----
            # masked-tile ets live from the score burst until the PV drain:
            # pool must hold max(masked)+L+2 or the in-order PE queue
            # deadlocks waiting on a buffer freed only later in the queue
            n_mask_max = max(
                sum(1 for jb in range(JBS)
                    if status[ib][jb] not in (SKIP, KEEP))
                for ib in range(IBS))
            with (
                tc.tile_pool(name="p2w", bufs=1) as p2w,
                tc.tile_pool(name="p2e", bufs=n_mask_max + 6) as p2e,
                tc.tile_pool(name="p2o", bufs=3) as p2o,
                tc.tile_pool(name="p2den", bufs=2) as p2den,
                tc.tile_pool(name="ps2s", bufs=2, space="PSUM") as ps2s,
                tc.tile_pool(name="ps2c", bufs=2, space="PSUM") as ps2c,
                tc.tile_pool(name="ps2o", bufs=2, space="PSUM") as ps2o,
            ):
                wo_sb = [p2w.tile([128, D], f16, name=f"wo{p}")
                         for p in range(2)]
                for p in range(2):
                    nc.sync.dma_start(
                        out=wo_sb[p], in_=wo_d[p * 128:(p + 1) * 128, :]
                    )
                mpat_sb = p2w.tile([128, n_pat, 512], f16)
                nc.sync.dma_start(
                    out=mpat_sb, in_=mpat_d.rearrange("n p f -> p n f")
                )

                pend = []  # deferred pair-finalize closures (cross-block)

                def emit_attention(ib, chunk_cb=None):
                    """scores+exp+PV for the 4 heads of i-block ib.

                    Masked (diagonal/pattern) tiles get their score matmul
                    FIRST and their PV matmul LAST, so the exp+mask chain on
                    ACT/Pool has the whole block's worth of PE work to hide
                    behind; unmasked tiles run a lead-L software pipeline.

                    Each head pair's softmax normalize is fused: 1/den rows
                    (PSUM partitions 64/32) -> fp16 -> PE partition-broadcast
                    -> one DVE multiply writing ctx16 straight from psc.
                    chunk_cb(k), called once per head, splices a slice of the
                    previous block's out-projection into this block so its
                    DVE copies never form a head-of-line block."""
                    masked = [jb for jb in range(JBS)
                              if status[ib][jb] not in (SKIP, KEEP)]
                    keeps = [jb for jb in range(JBS)
                             if status[ib][jb] == KEEP]
                    pv_order = keeps + masked    # accumulation emission
                    # psc's first accumulation must cover the full width
                    if not keeps:
                        full = [jb for jb in pv_order
                                if _affine_c0(ib, jb) == 0]
                        first = full[0] if full else pv_order[0]
                        pv_order.remove(first)
                        pv_order.insert(0, first)
                    dens = p2den.tile([65, 2, 512], f32, tag="dens")
                    d16 = p2den.tile([65, 2, 512], f16, tag="d16")
                    L = 3
                    ibb = slice(ib * 512, (ib + 1) * 512)
                    pscs = {}

                    def finalize(pair):
                        """1/den broadcast + fused normalize for both heads
                        of `pair` (deferred so PE work covers the chain)."""
                        nc.vector.tensor_copy(d16[64:65, pair, :],
                                              dens[64:65, pair, :])
                        nc.vector.tensor_copy(d16[32:33, pair, :],
                                              dens[32:33, pair, :])
                        dbc = ps2o.tile([128, 512], f32, tag="pso")
                        nc.tensor.matmul(dbc, ones65[64:65, :],
                                         d16[64:65, pair, :],
                                         start=True, stop=False)
                        nc.tensor.matmul(dbc, ones65[32:33, :],
                                         d16[32:33, pair, :],
                                         start=False, stop=True)
                        # DVE may read only one PSUM input: hop dbc to SBUF
                        dbs = p2den.tile([128, 512], f32, tag="dbs")
                        nc.vector.tensor_copy(dbs, dbc)
                        pe, po = pscs.pop((pair, 0)), pscs.pop((pair, 1))
                        nc.vector.tensor_mul(
                            ctx16[pair][0:64, ibb], pe[0:64, :],
                            dbs[0:64, :])
                        nc.vector.tensor_mul(
                            ctx16[pair][64:128, ibb], po[64:128, :],
                            dbs[64:128, :])

                    for pair in range(2):
                        psc_e = ps2c.tile([128, 512], f32, tag="psc")
                        psc_o = ps2c.tile([128, 512], f32, tag="psc")
                        pscs[(pair, 0)], pscs[(pair, 1)] = psc_e, psc_o
                        ets = {}

                        def emit_score(jb):
                            """both heads of the pair share kT: two score
                            matmuls into a 2-bank PSUM tile, ONE exp and ONE
                            mask op over [128, 2, w]."""
                            st = status[ib][jb]
                            c0 = _affine_c0(ib, jb) if st == AFFINE else 0
                            pss = ps2s.tile([128, 2, 512], f32, tag="pss")
                            # one matmul per head: a single 2-bank-spanning
                            # matmul fails the ISA check on HW
                            for s in range(2):
                                nc.tensor.matmul(
                                    pss[:, s, c0:512],
                                    kT[:, jb * 128:(jb + 1) * 128],
                                    qT[:, 2 * pair + s,
                                       ib * 512 + c0:(ib + 1) * 512],
                                    start=True, stop=True)
                            et = p2e.tile([128, 2, 512], f16, tag="et")
                            nc.scalar.activation(
                                et[:, :, c0:512], pss[:, :, c0:512], AF.Exp,
                                scale=inv_sqrt_d)
                            if st == AFFINE:
                                nc.gpsimd.affine_select(
                                    out=et[:, :, c0:512],
                                    in_=et[:, :, c0:512],
                                    compare_op=mybir.AluOpType.is_ge,
                                    fill=0.0,
                                    base=ib * 512 - jb * 128 + c0,
                                    pattern=[[0, 2], [1, 512 - c0]],
                                    channel_multiplier=-1,
                                )
                            elif isinstance(st, tuple):
                                nc.vector.tensor_mul(
                                    et, et,
                                    mpat_sb[:, st[1], :]
                                    .rearrange("p (o f) -> p o f", o=1)
                                    .to_broadcast([128, 2, 512]))
                            ets[jb] = (c0, et)

                        def emit_pv(m):
                            jb = pv_order[m]
                            c0, et = ets.pop(jb)
                            nc.tensor.matmul(
                                psc_e[0:65, c0:512],
                                v_aug[:, jb, :], et[:, 0, c0:512],
                                start=(m == 0),
                                stop=(m == len(pv_order) - 1))
                            nc.tensor.matmul(
                                psc_o[:, c0:512],
                                v_aug2[:, jb, :], et[:, 1, c0:512],
                                start=(m == 0),
                                stop=(m == len(pv_order) - 1))

                        # masked tiles' scores first (their exp+mask chains
                        # run while the keeps stream), then the keeps with a
                        # lead-L pipeline; masked PVs drain at the end
                        for jb in masked:
                            emit_score(jb)
                        while pend:  # prev pair's finalize, under PE cover
                            pend.pop()()
                        if chunk_cb is not None:
                            chunk_cb(2 * pair)
                        pv = 0
                        did_mid = False
                        for n, jb in enumerate(keeps):
                            emit_score(jb)
                            if n == len(keeps) // 2 and chunk_cb and \
                                    not did_mid:
                                chunk_cb(2 * pair + 1)
                                did_mid = True
                            if n >= L:
                                emit_pv(pv)
                                pv += 1
                        while pv < len(pv_order):
                            emit_pv(pv)
                            pv += 1
                        if chunk_cb is not None and not did_mid:
                            chunk_cb(2 * pair + 1)
                        # 1/den straight from PSUM (no partition shift)
                        nc.vector.reciprocal(
                            dens[64:65, pair, :], psc_e[64:65, :])
                        nc.vector.reciprocal(
                            dens[32:33, pair, :], psc_o[32:33, :])
                        pend.append(lambda p=pair: finalize(p))

                def emit_outproj_tt(tt, tail=False):
                    """one t-tile (128 rows) of the out-projection"""
                    ttb = slice(tt * 128, (tt + 1) * 128)
                    for cb in range(4):
                        cbb = slice(cb * 512, (cb + 1) * 512)
                        pso = ps2o.tile([128, 512], f32, tag="pso")
                        nc.tensor.matmul(pso, ctx16[0][:, ttb],
                                         wo_sb[0][:, cbb],
                                         start=True, stop=False)
                        nc.tensor.matmul(pso, ctx16[1][:, ttb],
                                         wo_sb[1][:, cbb],
                                         start=False, stop=True)
                        ot = p2o.tile([128, 512], f16, tag="ot")
                        # tail: ACT is idle (no more exps), split the drain;
                        # earlier: ACT copies would queue behind the next
                        # block's exps and hold the pso bank
                        if tail and cb % 2 == 1:
                            nc.scalar.activation(ot, pso, AF.Copy)
                        else:
                            nc.vector.tensor_copy(ot, pso)
                        nc.sync.dma_start(out=out_d[ttb, cbb], in_=ot)

                # attention(ib) runs one i-block ahead of the normalize +
                # out-projection so the PE never waits on the den bounce
                if _PARTS == "p1":
                    nc.sync.dma_start(out=out_d[0:64, 0:512],
                                      in_=kT[:, 0:512])
                else:
                    for ib in range(IBS):
                        cb = None
                        if ib > 0 and _PARTS == "full":
                            cb = (lambda k, base=(ib - 1) * 4:
                                  emit_outproj_tt(base + k))
                        emit_attention(ib, cb)
                    while pend:
                        pend.pop()()
                    if _PARTS == "full":
                        for k in range(4):
                            emit_outproj_tt((IBS - 1) * 4 + k, tail=True)

    _split_multiwaits(nc)
    return nc


_CACHE = {}


def _get_program(mask_key, status, n_pat, reps=1):
    key = (mask_key, reps)
    if key not in _CACHE:
        _CACHE[key] = _build_program(status, n_pat, reps)
    return _CACHE[key]


def _prepare(x, mask, cos, sin, W_query, W_key, W_value, W_out,
             q_scale, k_scale, reps=1):
    """Host-side prep: fold scales into rope tables, shard weights,
    classify the mask.  Returns (nc, in_maps)."""
    cos = np.asarray(cos, dtype=np.float32)
    sin = np.asarray(sin, dtype=np.float32)
    W_query = np.asarray(W_query, dtype=np.float32)
    W_key = np.asarray(W_key, dtype=np.float32)
    W_value = np.asarray(W_value, dtype=np.float32)
    W_out = np.asarray(W_out, dtype=np.float32)
    q_scale = np.asarray(q_scale, dtype=np.float32)
    k_scale = np.asarray(k_scale, dtype=np.float32)
    mask = np.asarray(mask)

    xf = np.ascontiguousarray(
        np.asarray(x).reshape(T, D).astype(np.float16)
    )

    # rope = qn*cos' + shuffle32(qn)*sin' with the rotate-half signs and the
    # post-norm q/k scales folded into the tables:
    #   rope(s*qn) = qn*(s*cos) + shuffle32(qn)*(shuffle32(s)*sin+-)
    def tables(scale):
        perm = np.concatenate([scale[HD // 2:], scale[:HD // 2]])
        c = (cos * scale[None, :]).astype(np.float32)
        s = (sin * perm[None, :]).astype(np.float32)
        s[:, :HD // 2] *= -1.0
        return np.ascontiguousarray(c), np.ascontiguousarray(s)

    cq, sq_t = tables(q_scale)
    ck, sk_t = tables(k_scale)
    cq, sq_t = cq.astype(np.float16), sq_t.astype(np.float16)
    ck, sk_t = ck.astype(np.float16), sk_t.astype(np.float16)

    status, patterns = _classify_mask(mask)
    nc = _get_program(mask.tobytes(), status, patterns.shape[0], reps)

    patterns = np.ascontiguousarray(patterns.astype(np.float16))
    in_maps = []
    for c in range(N_CORES):
        qcols = slice(c * H_LOC * HD, (c + 1) * H_LOC * HD)
        kvcols = slice(c * HD, (c + 1) * HD)
        wqkv = np.concatenate(
            [W_query[:, qcols], W_key[:, kvcols], W_value[:, kvcols]],
            axis=1).astype(np.float16)
        in_maps.append({
            "x": xf,
            "wqkv": np.ascontiguousarray(wqkv),
            "wo": np.ascontiguousarray(
                W_out[qcols, :].astype(np.float16)),
            "cosq": cq, "sinq": sq_t, "cosk": ck, "sink": sk_t,
            "mpat": patterns,
        })
    return nc, in_maps


def kernel(x, mask, cos, sin, W_query, W_key, W_value, W_out,
           q_scale, k_scale):
    out_dtype = np.asarray(x).dtype
    nc, in_maps = _prepare(x, mask, cos, sin, W_query, W_key, W_value,
                           W_out, q_scale, k_scale)

    from concourse.bass_utils import run_bass_kernel_spmd

    res = run_bass_kernel_spmd(nc, in_maps, list(range(N_CORES)))
    acc = res.results[0]["out"].astype(np.float32)
    for c in range(1, N_CORES):
        acc = acc + res.results[c]["out"].astype(np.float32)
    return acc.reshape(1, T, D).astype(out_dtype)


# revision 62
# speedup vs baseline: 1.0124x; 1.0124x over previous
"""Grouped-Query Attention kernel for 8 Trainium2 NeuronCores.

Reference model: x[1,2048,2048] -> Q(32 heads x 64) / K,V(8 kv heads x 64),
per-head RMS-norm(Q,K) + RoPE, causal softmax attention, out-projection.

Sharding (tensor-parallel over heads): core c owns Q heads 4c..4c+3 and KV
head c (exactly its GQA group) and W_out rows [256c : 256c+256).  Each core
computes a full-shape partial output; the host sums the 8 partials (the
unshard step for a row-sharded W_out).

On-core strategy (fp16 matmul path, ~3.3x over the fp32 version in the
cost model):
  - all matmul inputs are float16 (PE runs 1 cycle/row vs 4 for fp32);
    accumulation stays fp32 in PSUM.  fp16 keeps 10 mantissa bits, and
    RMS-norm bounds |q.k| <= 64 so exp(s/8) <= e^8 ~ 3e3 < fp16 max.
    Measured on HW: rel err ~6e-4 vs the fp32 reference.
  - x is loaded TRANSPOSED via the DMA XBAR (dma_start_transpose, 2-byte
    dtype, quarter-major order): no PE transposes / PSUM round-trips for x.
  - q/k/v projections run as ONE fused [128,384] PSUM accumulation; the
    rms-norm sum-of-squares runs on Pool (from an fp16 SBUF copy of q/k,
    exploiting that RoPE is norm-preserving is NOT assumed - sums are taken
    pre-rope), rope runs fp16 on DVE, and the per-tile PE transposes of
    q/k trail the projection by two tiles so the 4-engine chain never
    stalls the PE.
  - scores are built TRANSPOSED (S^T[j,i] = k_j . q_i) so that
      * PV needs no attention-matrix transpose
      * the softmax denominator comes free via an extra ones-column in V
  - causal diagonal tiles only compute the live column sub-range
  - the two heads of a GQA pair are processed together: one exp and one
    (3D) affine_select cover both heads' score tiles, halving ACT/Pool
    instruction counts
  - masked tiles' scores run first and their PVs drain last; unmasked
    tiles run a lead-3 software pipeline, so the PE never waits on exp
  - softmax normalize is fused: 1/den (DVE reciprocal from PSUM) -> fp16
    -> partition-broadcast via two 1-partition PE matmuls -> one DVE
    multiply writes normalized fp16 ctx straight from the PV accumulator
  - the out-projection is sliced per t-tile and interleaved into the NEXT
    attention block (between head pairs) so its PSUM->SBUF copies never
    head-of-line-block the DVE queue; partial outputs are stored fp16
    (host sums in fp32)
  - q/k scales and the rotate-half signs are folded into host-precomputed
    cos/sin tables
"""

import numpy as np

T = 2048
D = 2048
NUM_HEADS = 32
NUM_KV = 8
HD = 64
N_CORES = 8
H_LOC = NUM_HEADS // N_CORES  # 4 q heads per core
EPS = 1e-6

TT = T // 128   # 16 t-tiles of 128 rows
CC = D // 128   # 16 contraction chunks
IBS = T // 512  # 4 i-blocks of 512 query positions
JBS = T // 128  # 16 j-blocks of 128 key positions

KEEP = "keep"
SKIP = "skip"
AFFINE = "affine"


def _classify_mask(mask: np.ndarray):
    """Per (ib, jb) scoresT tile: how to apply the mask.

    Returns (status[IBS][JBS], patterns[n,128,512]) where patterns are
    multiplicative keep-masks in S^T (j, i) layout for irregular tiles.
    """
    keep = ~mask
    status = [[KEEP] * JBS for _ in range(IBS)]
    pat_index: dict[bytes, int] = {}
    pats: list[np.ndarray] = []
    ii, jj = np.meshgrid(np.arange(512), np.arange(128), indexing="ij")
    for ib in range(IBS):
        for jb in range(JBS):
            sub = keep[ib * 512:(ib + 1) * 512, jb * 128:(jb + 1) * 128]
            if sub.all():
                status[ib][jb] = KEEP
            elif not sub.any():
                status[ib][jb] = SKIP
            else:
                causal = (jb * 128 + jj) <= (ib * 512 + ii)
                if (sub == causal).all():
                    status[ib][jb] = AFFINE
                else:
                    key = sub.tobytes()
                    if key not in pat_index:
                        pat_index[key] = len(pats)
                        pats.append(sub.T.astype(np.float32))  # [128 j, 512 i]
                    status[ib][jb] = ("pat", pat_index[key])
    patterns = (
        np.stack(pats) if pats else np.zeros((1, 128, 512), dtype=np.float32)
    )
    return status, patterns


def _split_multiwaits(nc):
    """walrus in this container accepts only ONE sync-wait per instruction;
    hoist extra waits onto preceding same-engine NoOps (program order on the
    engine queue preserves the gating)."""
    import bass_rust
    from concourse import mybir

    n_fixed = 0
    for fn in nc.m.functions:
        for bb in fn.blocks:
            out = []
            for ins in bb.instructions:
                si = ins.sync_info
                if si is not None and si.on_wait and len(si.on_wait) > 1:
                    waits = list(si.on_wait)
                    ups = list(si.on_update) if si.on_update else []
                    for k, w in enumerate(waits[:-1]):
                        nop = mybir.InstNoOp(
                            name=f"{ins.name}-wnop{k}", ins=[], outs=[]
                        )
                        nop.engine = ins.engine
                        nop.sync_info = bass_rust.SyncInfo(
                            on_wait=[w], on_update=[]
                        )
                        out.append(nop)
                    ins.sync_info = bass_rust.SyncInfo(
                        on_wait=[waits[-1]], on_update=ups
                    )
                    n_fixed += 1
                out.append(ins)
            bb.instructions = out
    return n_fixed


def _affine_c0(ib, jb):
    """live-column start of an AFFINE (causal-diagonal) S^T tile: columns
    i_local < (jb - 4*ib)*128 are fully masked -> skip them entirely."""
    return max(0, min(3, jb - ib * (512 // 128))) * 128


_PARTS = "full"  # test-only knob: "p1" / "noout" / "full"


def _build_program(status, n_pat, reps=1):
    import concourse.bass as bass
    import concourse.mybir as mybir
    import concourse.tile as tile
    from concourse.masks import make_identity

    f32 = mybir.dt.float32
    f16 = mybir.dt.float16
    AF = mybir.ActivationFunctionType
    AX = mybir.AxisListType

    nc = bass.Bass("TRN2", num_devices=N_CORES)
    x_d = nc.declare_dram_parameter("x", [T, D], f16, isOutput=False)
    wqkv_d = nc.declare_dram_parameter(
        "wqkv", [D, (H_LOC + 2) * HD], f16, isOutput=False)
    wo_d = nc.declare_dram_parameter("wo", [H_LOC * HD, D], f16, isOutput=False)
    cosq_d = nc.declare_dram_parameter("cosq", [T, HD], f16, isOutput=False)
    sinq_d = nc.declare_dram_parameter("sinq", [T, HD], f16, isOutput=False)
    cosk_d = nc.declare_dram_parameter("cosk", [T, HD], f16, isOutput=False)
    sink_d = nc.declare_dram_parameter("sink", [T, HD], f16, isOutput=False)
    mpat_d = nc.declare_dram_parameter(
        "mpat", [n_pat, 128, 512], f16, isOutput=False
    )
    out_d = nc.declare_dram_parameter("out", [T, D], f16, isOutput=True)

    inv_sqrt_d = float(1.0 / np.sqrt(HD))

    with tile.TileContext(nc) as tc:
      for _rep in range(reps):
        with (
            tc.tile_pool(name="const", bufs=1) as const,
            tc.tile_pool(name="persist", bufs=1) as persist,
        ):
            ident = const.tile([128, 128], f16)
            make_identity(nc, ident)
            eps_t = const.tile([128, 1], f32)
            nc.vector.memset(eps_t, EPS)
            # partition-broadcast helpers: onesE spreads a [1,512] row to
            # out partitions 0:64, onesO to 64:128 (via PE matmul)
            # partition-broadcast helpers: engine ops must start at
            # partition 0/32/64/96 and stay in-window, and den rows live at
            # psc partitions 64 (even head) / 32 (odd).  Two accumulating
            # 1-partition matmuls spread row64 -> out 0:64, row32 -> 64:128.
            ones65 = const.tile([65, 128], f16, name="ones65")
            nc.vector.memset(ones65[32:33, :], 0.0)
            nc.vector.memset(ones65[64:65, :], 0.0)
            nc.vector.memset(ones65[64:65, 0:64], 1.0)
            nc.vector.memset(ones65[32:33, 64:128], 1.0)

            # persistent across phases
            qT = persist.tile([64, H_LOC, T], f16)
            kT = persist.tile([64, T], f16)
            # v with aux columns:
            #  v_aug  [128,TT,65]:  cols 0:64 = v, col 64 = 1 (even head)
            #  v_aug2 [128,TT,128]: col 32 = 1, cols 64:128 = v (odd head)
            v_aug = persist.tile([128, TT, 65], f16)
            nc.vector.memset(v_aug[:, :, 64:65], 1.0)
            v_aug2 = persist.tile([128, TT, 128], f16)
            nc.vector.memset(v_aug2[:, :, 0:64], 0.0)
            nc.vector.memset(v_aug2[:, :, 32:33], 1.0)
            # normalized fp16 ctx^T (written straight from PSUM by the
            # fused 1/den multiply), consumed by the out-projection
            ctx16 = [persist.tile([128, T], f16, name=f"ctx16{p}")
                     for p in range(2)]

            # ---- phase 1: load x^T, project q/k/v, rms-norm + rope ----
            with (
                tc.tile_pool(name="p1w", bufs=1) as p1w,
                tc.tile_pool(name="p1t", bufs=4) as p1t,
                tc.tile_pool(name="ps1a", bufs=2, space="PSUM") as ps1a,
                tc.tile_pool(name="ps1b", bufs=3, space="PSUM") as ps1b,
            ):
                MQKV = (H_LOC + 2) * HD  # 384: q heads | k | v
                # load in 4 cc-range chunks so the first projection matmul
                # starts after ~1/4 of the weight traffic
                wqkv_sb = p1w.tile([128, CC, MQKV], f16)
                wqkv_r = wqkv_d.rearrange("(cc p) m -> p cc m", p=128)
                for wq4 in range(4):
                    cs = slice(wq4 * (CC // 4), (wq4 + 1) * (CC // 4))
                    nc.sync.dma_start(
                        out=wqkv_sb[:, cs, :], in_=wqkv_r[:, cs, :])
                ctabs = {}
                for nm, dd in (("cosq", cosq_d), ("sinq", sinq_d),
                               ("cosk", cosk_d), ("sink", sink_d)):
                    tab = p1w.tile([128, TT, HD], f16, name=f"tab_{nm}")
                    nc.sync.dma_start(
                        out=tab, in_=dd.rearrange("(tt p) d -> p tt d", p=128)
                    )
                    ctabs[nm] = tab

                # x transposed via the DMA XBAR:
                #   xt_all[p, cc, t] = x[t, cc*128 + p]
                # split in t-quarters, quarter-major, so the tt=0..3
                # projections can start after ~1/4 of the transpose traffic
                xt_all = p1w.tile([128, CC, T], f16)
                for th in range(4):
                    tb = slice(th * (T // 4), (th + 1) * (T // 4))
                    for cc in range(CC):
                        nc.sync.dma_start_transpose(
                            out=xt_all[:, cc, tb],
                            in_=x_d[tb, cc * 128:(cc + 1) * 128],
                        )

                def emit_proj(tt, mid=None):
                    ps = ps1b.tile([128, MQKV], f32, tag="psqkv")
                    for cc in range(CC):
                        if cc == CC // 2 and mid is not None:
                            mid()  # splice transposes mid-accumulation
                        nc.tensor.matmul(
                            ps, xt_all[:, cc, tt * 128:(tt + 1) * 128],
                            wqkv_sb[:, cc, :],
                            start=(cc == 0), stop=(cc == CC - 1))
                    return ps

                def emit_rope(tt, ps):
                    """rms-norm + rope; q/k hop PSUM->SBUF fp16 once (ACT),
                    then all elementwise work runs fp16.  Returns (qr, kr)."""
                    QK = (H_LOC + 1) * HD  # q heads + k
                    qk16 = p1t.tile([128, QK], f16, tag="qk16")
                    nc.scalar.activation(qk16, ps[:, 0:QK], AF.Copy)
                    # v copies: ACT from PSUM, then Pool SBUF->SBUF
                    nc.scalar.activation(v_aug[:, tt, 0:64],
                                         ps[:, QK:QK + HD], AF.Copy)
                    nc.gpsimd.tensor_copy(v_aug2[:, tt, 64:128],
                                          v_aug[:, tt, 0:64])

                    # sum of squares per head (Pool), 1/rms (ACT sqrt + DVE)
                    q5 = qk16.rearrange("p (h d) -> p h d", h=H_LOC + 1)
                    sq16 = p1t.tile([128, H_LOC + 1, HD], f16, tag="sq16")
                    nc.gpsimd.tensor_mul(sq16, q5, q5)
                    ssk = p1t.tile([128, H_LOC + 1, 1], f32, tag="ssk")
                    nc.vector.reduce_sum(ssk, sq16, axis=AX.X)
                    rinv = p1t.tile([128, H_LOC + 1], f32, tag="rinv")
                    nc.scalar.activation(
                        rinv, ssk.rearrange("p h o -> p (h o)"), AF.Sqrt,
                        bias=eps_t[:, 0:1], scale=1.0 / HD)
                    nc.vector.reciprocal(rinv, rinv)
                    r16 = p1t.tile([128, H_LOC + 1, 1], f16, tag="r16")
                    nc.vector.tensor_copy(
                        r16, rinv.rearrange("p (h o) -> p h o", o=1))

                    # rope(q) * rinv_q (fp16 all the way; rms-norm commutes
                    # with rope so the 1/rms multiply comes last)
                    q3 = q5[:, 0:H_LOC, :]
                    cq = ctabs["cosq"][:, tt, :].rearrange(
                        "p (o d) -> p o d", o=1)
                    sq = ctabs["sinq"][:, tt, :].rearrange(
                        "p (o d) -> p o d", o=1)
                    qr_f = p1t.tile([128, H_LOC, HD], f16, tag="qr_f")
                    nc.vector.tensor_mul(
                        qr_f, q3, cq.to_broadcast([128, H_LOC, HD]))
                    qrot = p1t.tile([128, H_LOC, HD], f16, tag="qrot")
                    nc.vector.tensor_mul(
                        qrot[:, :, 0:32], q3[:, :, 32:64],
                        sq[:, :, 0:32].to_broadcast([128, H_LOC, 32]))
                    nc.vector.tensor_mul(
                        qrot[:, :, 32:64], q3[:, :, 0:32],
                        sq[:, :, 32:64].to_broadcast([128, H_LOC, 32]))
                    nc.vector.tensor_add(qr_f, qr_f, qrot)
                    qr = p1t.tile([128, H_LOC, HD], f16, tag="qr")
                    nc.vector.tensor_mul(
                        qr, qr_f,
                        r16[:, 0:H_LOC, :].to_broadcast([128, H_LOC, HD]))

                    # rope(k) * rinv_k
                    k1 = q5[:, H_LOC, :]
                    kr_f = p1t.tile([128, HD], f16, tag="kr_f")
                    nc.vector.tensor_mul(kr_f, k1, ctabs["cosk"][:, tt, :])
                    krot = p1t.tile([128, HD], f16, tag="krot")
                    nc.vector.tensor_mul(
                        krot[:, 0:32], k1[:, 32:64],
                        ctabs["sink"][:, tt, 0:32])
                    nc.vector.tensor_mul(
                        krot[:, 32:64], k1[:, 0:32],
                        ctabs["sink"][:, tt, 32:64])
                    nc.vector.tensor_add(kr_f, kr_f, krot)
                    kr = p1t.tile([128, HD], f16, tag="kr")
                    nc.vector.tensor_mul(
                        kr, kr_f,
                        r16[:, H_LOC, :].to_broadcast([128, HD]))
                    return qr, kr

                def emit_transpose(tt, qr, kr):
                    psqt = ps1a.tile([64, 512], f16, tag="psqt")
                    for h in range(H_LOC):
                        nc.tensor.transpose(
                            psqt[:, h * 128:(h + 1) * 128], qr[:, h, :],
                            ident)
                    # one strided DVE copy: psqt [64,(4,128)] -> qT[:,h,ttb]
                    nc.vector.tensor_copy(
                        qT[:, :, tt * 128:(tt + 1) * 128],
                        psqt.rearrange("p (h t) -> p h t", h=H_LOC))
                    pskt = ps1a.tile([64, 128], f16, tag="pskt")
                    nc.tensor.transpose(pskt, kr, ident)
                    nc.vector.tensor_copy(
                        kT[:, tt * 128:(tt + 1) * 128], pskt)

                # software-pipeline: transposes(tt-1) are emitted after
                # proj(tt) so the PE never waits on the DVE rope
                # transposes run TWO t-tiles behind the projection: the
                # rms+rope chain spans ~4 engines and needs ~2 proj-tiles
                # of PE time to finish without stalling the transposes
                from collections import deque
                lag = deque()
                for tt in range(TT):
                    mid = None
                    if len(lag) >= 2:
                        mid = (lambda p=lag.popleft():
                               emit_transpose(*p))
                    ps = emit_proj(tt, mid)
                    lag.append((tt,) + emit_rope(tt, ps))
                while lag:
                    emit_transpose(*lag.popleft())

            # ---- phase 2+3: attention + out-projection ----
            # masked-tile ets live from the score burst until the PV drain:
            # pool must hold max(masked)+L+2 or the in-order PE queue
            # deadlocks waiting on a buffer freed only later in the queue
            n_mask_max = max(
                sum(1 for jb in range(JBS)
                    if status[ib][jb] not in (SKIP, KEEP))
                for ib in range(IBS))
            with (
                tc.tile_pool(name="p2w", bufs=1) as p2w,
                tc.tile_pool(name="p2e", bufs=n_mask_max + 6) as p2e,
                tc.tile_pool(name="p2o", bufs=3) as p2o,
                tc.tile_pool(name="p2den", bufs=2) as p2den,
                tc.tile_pool(name="ps2s", bufs=2, space="PSUM") as ps2s,
                tc.tile_pool(name="ps2c", bufs=2, space="PSUM") as ps2c,
                tc.tile_pool(name="ps2o", bufs=2, space="PSUM") as ps2o,
            ):
                wo_sb = [p2w.tile([128, D], f16, name=f"wo{p}")
                         for p in range(2)]
                for p in range(2):
                    nc.sync.dma_start(
                        out=wo_sb[p], in_=wo_d[p * 128:(p + 1) * 128, :]
                    )
                mpat_sb = p2w.tile([128, n_pat, 512], f16)
                nc.sync.dma_start(
                    out=mpat_sb, in_=mpat_d.rearrange("n p f -> p n f")
                )

                pend = []  # deferred pair-finalize closures (cross-block)

                def emit_attention(ib, chunk_cb=None):
                    """scores+exp+PV for the 4 heads of i-block ib.

                    Masked (diagonal/pattern) tiles get their score matmul
                    FIRST and their PV matmul LAST, so the exp+mask chain on
                    ACT/Pool has the whole block's worth of PE work to hide
                    behind; unmasked tiles run a lead-L software pipeline.

                    Each head pair's softmax normalize is fused: 1/den rows
                    (PSUM partitions 64/32) -> fp16 -> PE partition-broadcast
                    -> one DVE multiply writing ctx16 straight from psc.
                    chunk_cb(k), called once per head, splices a slice of the
                    previous block's out-projection into this block so its
                    DVE copies never form a head-of-line block."""
                    masked = [jb for jb in range(JBS)
                              if status[ib][jb] not in (SKIP, KEEP)]
                    keeps = [jb for jb in range(JBS)
                             if status[ib][jb] == KEEP]
                    pv_order = keeps + masked    # accumulation emission
                    # psc's first accumulation must cover the full width
                    if not keeps:
                        full = [jb for jb in pv_order
                                if _affine_c0(ib, jb) == 0]
                        first = full[0] if full else pv_order[0]
                        pv_order.remove(first)
                        pv_order.insert(0, first)
                    dens = p2den.tile([65, 2, 512], f32, tag="dens")
                    d16 = p2den.tile([65, 2, 512], f16, tag="d16")
                    L = 3
                    ibb = slice(ib * 512, (ib + 1) * 512)
                    pscs = {}

                    def finalize(pair):
                        """1/den broadcast + fused normalize for both heads
                        of `pair` (deferred so PE work covers the chain)."""
                        nc.vector.tensor_copy(d16[64:65, pair, :],
                                              dens[64:65, pair, :])
                        nc.vector.tensor_copy(d16[32:33, pair, :],
                                              dens[32:33, pair, :])
                        dbc = ps2o.tile([128, 512], f32, tag="pso")
                        nc.tensor.matmul(dbc, ones65[64:65, :],
                                         d16[64:65, pair, :],
                                         start=True, stop=False)
                        nc.tensor.matmul(dbc, ones65[32:33, :],
                                         d16[32:33, pair, :],
                                         start=False, stop=True)
                        # DVE may read only one PSUM input: hop dbc to SBUF
                        dbs = p2den.tile([128, 512], f32, tag="dbs")
                        nc.vector.tensor_copy(dbs, dbc)
                        pe, po = pscs.pop((pair, 0)), pscs.pop((pair, 1))
                        nc.vector.tensor_mul(
                            ctx16[pair][0:64, ibb], pe[0:64, :],
                            dbs[0:64, :])
                        nc.vector.tensor_mul(
                            ctx16[pair][64:128, ibb], po[64:128, :],
                            dbs[64:128, :])

                    for pair in range(2):
                        psc_e = ps2c.tile([128, 512], f32, tag="psc")
                        psc_o = ps2c.tile([128, 512], f32, tag="psc")
                        pscs[(pair, 0)], pscs[(pair, 1)] = psc_e, psc_o
                        ets = {}

                        def emit_score(jb):
                            """both heads of the pair share kT: two score
                            matmuls into a 2-bank PSUM tile, ONE exp and ONE
                            mask op over [128, 2, w]."""
                            st = status[ib][jb]
                            c0 = _affine_c0(ib, jb) if st == AFFINE else 0
                            pss = ps2s.tile([128, 2, 512], f32, tag="pss")
                            # one matmul per head: a single 2-bank-spanning
                            # matmul fails the ISA check on HW
                            for s in range(2):
                                nc.tensor.matmul(
                                    pss[:, s, c0:512],
                                    kT[:, jb * 128:(jb + 1) * 128],
                                    qT[:, 2 * pair + s,
                                       ib * 512 + c0:(ib + 1) * 512],
                                    start=True, stop=True)
                            et = p2e.tile([128, 2, 512], f16, tag="et")
                            nc.scalar.activation(
                                et[:, :, c0:512], pss[:, :, c0:512], AF.Exp,
                                scale=inv_sqrt_d)
                            if st == AFFINE:
                                nc.gpsimd.affine_select(
                                    out=et[:, :, c0:512],
                                    in_=et[:, :, c0:512],
                                    compare_op=mybir.AluOpType.is_ge,
                                    fill=0.0,
                                    base=ib * 512 - jb * 128 + c0,
                                    pattern=[[0, 2], [1, 512 - c0]],
                                    channel_multiplier=-1,
                                )
                            elif isinstance(st, tuple):
                                nc.vector.tensor_mul(
                                    et, et,
                                    mpat_sb[:, st[1], :]
                                    .rearrange("p (o f) -> p o f", o=1)
                                    .to_broadcast([128, 2, 512]))
                            ets[jb] = (c0, et)

                        def emit_pv(m):
                            jb = pv_order[m]
                            c0, et = ets.pop(jb)
                            nc.tensor.matmul(
                                psc_e[0:65, c0:512],
                                v_aug[:, jb, :], et[:, 0, c0:512],
                                start=(m == 0),
                                stop=(m == len(pv_order) - 1))
                            nc.tensor.matmul(
                                psc_o[:, c0:512],
                                v_aug2[:, jb, :], et[:, 1, c0:512],
                                start=(m == 0),
                                stop=(m == len(pv_order) - 1))

                        # masked tiles' scores first: while the PE stalls on
                        # the 2-deep pss pool, ACT front-loads the slow
                        # masked exps and the keeps then pipeline cleanly
                        # (interleaving measures worse); PVs drain
                        # keeps-first, masked-last with a lead-L pipeline
                        for jb in masked:
                            emit_score(jb)
                        while pend:  # prev pair's finalize, under PE cover
                            pend.pop()()
                        if chunk_cb is not None:
                            chunk_cb(2 * pair)
                        pv = 0
                        did_mid = False
                        for n, jb in enumerate(keeps):
                            emit_score(jb)
                            if n == len(keeps) // 2 and chunk_cb and \
                                    not did_mid:
                                chunk_cb(2 * pair + 1)
                                did_mid = True
                            if n >= L:
                                emit_pv(pv)
                                pv += 1
                        while pv < len(pv_order):
                            emit_pv(pv)
                            pv += 1
                        if chunk_cb is not None and not did_mid:
                            chunk_cb(2 * pair + 1)
                        # 1/den straight from PSUM (no partition shift)
                        nc.vector.reciprocal(
                            dens[64:65, pair, :], psc_e[64:65, :])
                        nc.vector.reciprocal(
                            dens[32:33, pair, :], psc_o[32:33, :])
                        pend.append(lambda p=pair: finalize(p))

                def emit_outproj_tt(tt, tail=False):
                    """one t-tile (128 rows) of the out-projection"""
                    ttb = slice(tt * 128, (tt + 1) * 128)
                    for cb in range(4):
                        cbb = slice(cb * 512, (cb + 1) * 512)
                        pso = ps2o.tile([128, 512], f32, tag="pso")
                        nc.tensor.matmul(pso, ctx16[0][:, ttb],
                                         wo_sb[0][:, cbb],
                                         start=True, stop=False)
                        nc.tensor.matmul(pso, ctx16[1][:, ttb],
                                         wo_sb[1][:, cbb],
                                         start=False, stop=True)
                        ot = p2o.tile([128, 512], f16, tag="ot")
                        # tail: ACT is idle (no more exps), split the drain;
                        # earlier: ACT copies would queue behind the next
                        # block's exps and hold the pso bank
                        if tail and cb % 2 == 1:
                            nc.scalar.activation(ot, pso, AF.Copy)
                        else:
                            nc.vector.tensor_copy(ot, pso)
                        nc.sync.dma_start(out=out_d[ttb, cbb], in_=ot)

                # attention(ib) runs one i-block ahead of the normalize +
                # out-projection so the PE never waits on the den bounce
                if _PARTS == "p1":
                    nc.sync.dma_start(out=out_d[0:64, 0:512],
                                      in_=kT[:, 0:512])
                else:
                    # ascending: block ib's out-projection (constant size)
                    # hides under attention(ib+1), which is always larger
                    order = list(range(IBS))
                    prev_ib = None
                    for ib in order:
                        cb = None
                        if prev_ib is not None and _PARTS == "full":
                            cb = (lambda k, base=prev_ib * 4:
                                  emit_outproj_tt(base + k))
                        emit_attention(ib, cb)
                        prev_ib = ib
                    while pend:
                        pend.pop()()
                    if _PARTS == "full":
                        for k in range(4):
                            emit_outproj_tt(prev_ib * 4 + k, tail=True)

    _split_multiwaits(nc)
    return nc


_CACHE = {}


def _get_program(mask_key, status, n_pat, reps=1):
    key = (mask_key, reps)
    if key not in _CACHE:
        _CACHE[key] = _build_program(status, n_pat, reps)
    return _CACHE[key]


def _prepare(x, mask, cos, sin, W_query, W_key, W_value, W_out,
             q_scale, k_scale, reps=1):
    """Host-side prep: fold scales into rope tables, shard weights,
    classify the mask.  Returns (nc, in_maps)."""
    cos = np.asarray(cos, dtype=np.float32)
    sin = np.asarray(sin, dtype=np.float32)
    W_query = np.asarray(W_query, dtype=np.float32)
    W_key = np.asarray(W_key, dtype=np.float32)
    W_value = np.asarray(W_value, dtype=np.float32)
    W_out = np.asarray(W_out, dtype=np.float32)
    q_scale = np.asarray(q_scale, dtype=np.float32)
    k_scale = np.asarray(k_scale, dtype=np.float32)
    mask = np.asarray(mask)

    xf = np.ascontiguousarray(
        np.asarray(x).reshape(T, D).astype(np.float16)
    )

    # rope = qn*cos' + shuffle32(qn)*sin' with the rotate-half signs and the
    # post-norm q/k scales folded into the tables:
    #   rope(s*qn) = qn*(s*cos) + shuffle32(qn)*(shuffle32(s)*sin+-)
    def tables(scale):
        perm = np.concatenate([scale[HD // 2:], scale[:HD // 2]])
        c = (cos * scale[None, :]).astype(np.float32)
        s = (sin * perm[None, :]).astype(np.float32)
        s[:, :HD // 2] *= -1.0
        return np.ascontiguousarray(c), np.ascontiguousarray(s)

    cq, sq_t = tables(q_scale)
    ck, sk_t = tables(k_scale)
    cq, sq_t = cq.astype(np.float16), sq_t.astype(np.float16)
    ck, sk_t = ck.astype(np.float16), sk_t.astype(np.float16)

    status, patterns = _classify_mask(mask)
    nc = _get_program(mask.tobytes(), status, patterns.shape[0], reps)

    patterns = np.ascontiguousarray(patterns.astype(np.float16))
    in_maps = []
    for c in range(N_CORES):
        qcols = slice(c * H_LOC * HD, (c + 1) * H_LOC * HD)
        kvcols = slice(c * HD, (c + 1) * HD)
        wqkv = np.concatenate(
            [W_query[:, qcols], W_key[:, kvcols], W_value[:, kvcols]],
            axis=1).astype(np.float16)
        in_maps.append({
            "x": xf,
            "wqkv": np.ascontiguousarray(wqkv),
            "wo": np.ascontiguousarray(
                W_out[qcols, :].astype(np.float16)),
            "cosq": cq, "sinq": sq_t, "cosk": ck, "sink": sk_t,
            "mpat": patterns,
        })
    return nc, in_maps


def kernel(x, mask, cos, sin, W_query, W_key, W_value, W_out,
           q_scale, k_scale):
    out_dtype = np.asarray(x).dtype
    nc, in_maps = _prepare(x, mask, cos, sin, W_query, W_key, W_value,
                           W_out, q_scale, k_scale)

    from concourse.bass_utils import run_bass_kernel_spmd

    res = run_bass_kernel_spmd(nc, in_maps, list(range(N_CORES)))
    acc = res.results[0]["out"].astype(np.float32)
    for c in range(1, N_CORES):
        acc = acc + res.results[c]["out"].astype(np.float32)
    return acc.reshape(1, T, D).astype(out_dtype)


# revision 63
# speedup vs baseline: 1.5671x; 1.5479x over previous
"""Grouped-Query Attention kernel for 8 Trainium2 NeuronCores.

Reference model: x[1,2048,2048] -> Q(32 heads x 64) / K,V(8 kv heads x 64),
per-head RMS-norm(Q,K) + RoPE, causal softmax attention, out-projection.

Sharding (tensor-parallel over heads): core c owns Q heads 4c..4c+3 and KV
head c (exactly its GQA group) and W_out rows [256c : 256c+256).  Each core
computes a full-shape partial output; the host sums the 8 partials (the
unshard step for a row-sharded W_out).

On-core strategy (fp16 matmul path, ~3.3x over the fp32 version in the
cost model):
  - all matmul inputs are float16 (PE runs 1 cycle/row vs 4 for fp32);
    accumulation stays fp32 in PSUM.  fp16 keeps 10 mantissa bits, and
    RMS-norm bounds |q.k| <= 64 so exp(s/8) <= e^8 ~ 3e3 < fp16 max.
    Measured on HW: rel err ~6e-4 vs the fp32 reference.
  - x is loaded TRANSPOSED via the DMA XBAR (dma_start_transpose, 2-byte
    dtype, quarter-major order): no PE transposes / PSUM round-trips for x.
  - q/k/v projections run as ONE fused [128,384] PSUM accumulation; the
    rms-norm sum-of-squares runs on Pool (from an fp16 SBUF copy of q/k,
    exploiting that RoPE is norm-preserving is NOT assumed - sums are taken
    pre-rope), rope runs fp16 on DVE, and the per-tile PE transposes of
    q/k trail the projection by two tiles so the 4-engine chain never
    stalls the PE.
  - scores are built TRANSPOSED (S^T[j,i] = k_j . q_i) so that
      * PV needs no attention-matrix transpose
      * the softmax denominator comes free via an extra ones-column in V
  - causal diagonal tiles only compute the live column sub-range
  - the two heads of a GQA pair are processed together: one exp and one
    (3D) affine_select cover both heads' score tiles, halving ACT/Pool
    instruction counts
  - masked tiles' scores run first and their PVs drain last; unmasked
    tiles run a lead-3 software pipeline, so the PE never waits on exp
  - softmax normalize is fused: 1/den (DVE reciprocal from PSUM) -> fp16
    -> partition-broadcast via two 1-partition PE matmuls -> one DVE
    multiply writes normalized fp16 ctx straight from the PV accumulator
  - the out-projection is sliced per t-tile and interleaved into the NEXT
    attention block (between head pairs) so its PSUM->SBUF copies never
    head-of-line-block the DVE queue; partial outputs are stored fp16
    (host sums in fp32)
  - q/k scales and the rotate-half signs are folded into host-precomputed
    cos/sin tables
"""

import numpy as np

T = 2048
D = 2048
NUM_HEADS = 32
NUM_KV = 8
HD = 64
N_CORES = 8
H_LOC = NUM_HEADS // N_CORES  # 4 q heads per core
EPS = 1e-6

TT = T // 128   # 16 t-tiles of 128 rows
CC = D // 128   # 16 contraction chunks
IBS = T // 512  # 4 i-blocks of 512 query positions
JBS = T // 128  # 16 j-blocks of 128 key positions

KEEP = "keep"
SKIP = "skip"
AFFINE = "affine"


def _classify_mask(mask: np.ndarray):
    """Per (ib, jb) scoresT tile: how to apply the mask.

    Returns (status[IBS][JBS], patterns[n,128,512]) where patterns are
    multiplicative keep-masks in S^T (j, i) layout for irregular tiles.
    """
    keep = ~mask
    status = [[KEEP] * JBS for _ in range(IBS)]
    pat_index: dict[bytes, int] = {}
    pats: list[np.ndarray] = []
    ii, jj = np.meshgrid(np.arange(512), np.arange(128), indexing="ij")
    for ib in range(IBS):
        for jb in range(JBS):
            sub = keep[ib * 512:(ib + 1) * 512, jb * 128:(jb + 1) * 128]
            if sub.all():
                status[ib][jb] = KEEP
            elif not sub.any():
                status[ib][jb] = SKIP
            else:
                causal = (jb * 128 + jj) <= (ib * 512 + ii)
                if (sub == causal).all():
                    status[ib][jb] = AFFINE
                else:
                    key = sub.tobytes()
                    if key not in pat_index:
                        pat_index[key] = len(pats)
                        pats.append(sub.T.astype(np.float32))  # [128 j, 512 i]
                    status[ib][jb] = ("pat", pat_index[key])
    patterns = (
        np.stack(pats) if pats else np.zeros((1, 128, 512), dtype=np.float32)
    )
    return status, patterns


def _split_multiwaits(nc):
    """walrus in this container accepts only ONE sync-wait per instruction;
    hoist extra waits onto preceding same-engine NoOps (program order on the
    engine queue preserves the gating)."""
    import bass_rust
    from concourse import mybir

    n_fixed = 0
    for fn in nc.m.functions:
        for bb in fn.blocks:
            out = []
            for ins in bb.instructions:
                si = ins.sync_info
                if si is not None and si.on_wait and len(si.on_wait) > 1:
                    waits = list(si.on_wait)
                    ups = list(si.on_update) if si.on_update else []
                    for k, w in enumerate(waits[:-1]):
                        nop = mybir.InstNoOp(
                            name=f"{ins.name}-wnop{k}", ins=[], outs=[]
                        )
                        nop.engine = ins.engine
                        nop.sync_info = bass_rust.SyncInfo(
                            on_wait=[w], on_update=[]
                        )
                        out.append(nop)
                    ins.sync_info = bass_rust.SyncInfo(
                        on_wait=[waits[-1]], on_update=ups
                    )
                    n_fixed += 1
                out.append(ins)
            bb.instructions = out
    return n_fixed


def _affine_c0(ib, jb):
    """live-column start of an AFFINE (causal-diagonal) S^T tile: columns
    i_local < (jb - 4*ib)*128 are fully masked -> skip them entirely."""
    return max(0, min(3, jb - ib * (512 // 128))) * 128


_PARTS = "full"  # test-only knob: "p1" / "noout" / "full"


def _build_program(status, n_pat, reps=1):
    import concourse.bass as bass
    import concourse.mybir as mybir
    import concourse.tile as tile
    from concourse.masks import make_identity

    f32 = mybir.dt.float32
    f16 = mybir.dt.float16
    AF = mybir.ActivationFunctionType
    AX = mybir.AxisListType

    nc = bass.Bass("TRN2", num_devices=N_CORES)
    x_d = nc.declare_dram_parameter("x", [T, D], f16, isOutput=False)
    wqkv_d = nc.declare_dram_parameter(
        "wqkv", [D, (H_LOC + 2) * HD], f16, isOutput=False)
    wo_d = nc.declare_dram_parameter("wo", [H_LOC * HD, D], f16, isOutput=False)
    cosq_d = nc.declare_dram_parameter("cosq", [T, HD], f16, isOutput=False)
    sinq_d = nc.declare_dram_parameter("sinq", [T, HD], f16, isOutput=False)
    cosk_d = nc.declare_dram_parameter("cosk", [T, HD], f16, isOutput=False)
    sink_d = nc.declare_dram_parameter("sink", [T, HD], f16, isOutput=False)
    mpat_d = nc.declare_dram_parameter(
        "mpat", [n_pat, 128, 512], f16, isOutput=False
    )
    out_d = nc.declare_dram_parameter("out", [T, D], f16, isOutput=True)

    inv_sqrt_d = float(1.0 / np.sqrt(HD))

    with tile.TileContext(nc) as tc:
      for _rep in range(reps):
        with (
            tc.tile_pool(name="const", bufs=1) as const,
            tc.tile_pool(name="persist", bufs=1) as persist,
        ):
            ident = const.tile([128, 128], f16)
            make_identity(nc, ident)
            eps_t = const.tile([128, 1], f32)
            nc.vector.memset(eps_t, EPS)
            # partition-broadcast helpers: onesE spreads a [1,512] row to
            # out partitions 0:64, onesO to 64:128 (via PE matmul)
            # partition-broadcast helpers: engine ops must start at
            # partition 0/32/64/96 and stay in-window, and den rows live at
            # psc partitions 64 (even head) / 32 (odd).  Two accumulating
            # 1-partition matmuls spread row64 -> out 0:64, row32 -> 64:128.
            ones65 = const.tile([65, 128], f16, name="ones65")
            nc.vector.memset(ones65[32:33, :], 0.0)
            nc.vector.memset(ones65[64:65, :], 0.0)
            nc.vector.memset(ones65[64:65, 0:64], 1.0)
            nc.vector.memset(ones65[32:33, 64:128], 1.0)

            # persistent across phases
            qT = persist.tile([64, H_LOC, T], f16)
            kT = persist.tile([64, T], f16)
            # v with aux columns:
            #  v_aug  [128,TT,65]:  cols 0:64 = v, col 64 = 1 (even head)
            #  v_aug2 [128,TT,128]: col 32 = 1, cols 64:128 = v (odd head)
            v_aug = persist.tile([128, TT, 65], f16)
            nc.vector.memset(v_aug[:, :, 64:65], 1.0)
            v_aug2 = persist.tile([128, TT, 128], f16)
            nc.vector.memset(v_aug2[:, :, 0:64], 0.0)
            nc.vector.memset(v_aug2[:, :, 32:33], 1.0)
            # normalized fp16 ctx^T (written straight from PSUM by the
            # fused 1/den multiply), consumed by the out-projection
            ctx16 = [persist.tile([128, T], f16, name=f"ctx16{p}")
                     for p in range(2)]

            # ---- phase 1: load x^T, project q/k/v, rms-norm + rope ----
            with (
                tc.tile_pool(name="p1w", bufs=1) as p1w,
                tc.tile_pool(name="p1t", bufs=4) as p1t,
                tc.tile_pool(name="ps1a", bufs=2, space="PSUM") as ps1a,
                tc.tile_pool(name="ps1b", bufs=3, space="PSUM") as ps1b,
            ):
                MQKV = (H_LOC + 2) * HD  # 384: q heads | k | v
                # load in 4 cc-range chunks so the first projection matmul
                # starts after ~1/4 of the weight traffic
                wqkv_sb = p1w.tile([128, CC, MQKV], f16)
                wqkv_r = wqkv_d.rearrange("(cc p) m -> p cc m", p=128)
                for wq4 in range(4):
                    cs = slice(wq4 * (CC // 4), (wq4 + 1) * (CC // 4))
                    nc.sync.dma_start(
                        out=wqkv_sb[:, cs, :], in_=wqkv_r[:, cs, :])
                ctabs = {}
                for nm, dd in (("cosq", cosq_d), ("sinq", sinq_d),
                               ("cosk", cosk_d), ("sink", sink_d)):
                    tab = p1w.tile([128, TT, HD], f16, name=f"tab_{nm}")
                    nc.sync.dma_start(
                        out=tab, in_=dd.rearrange("(tt p) d -> p tt d", p=128)
                    )
                    ctabs[nm] = tab

                # x transposed via the DMA XBAR:
                #   xt_all[p, cc, t] = x[t, cc*128 + p]
                # split in t-quarters, quarter-major, so the tt=0..3
                # projections can start after ~1/4 of the transpose traffic
                xt_all = p1w.tile([128, CC, T], f16)
                for th in range(4):
                    tb = slice(th * (T // 4), (th + 1) * (T // 4))
                    for cc in range(CC):
                        nc.sync.dma_start_transpose(
                            out=xt_all[:, cc, tb],
                            in_=x_d[tb, cc * 128:(cc + 1) * 128],
                        )

                def emit_proj(tt, mid=None):
                    ps = ps1b.tile([128, MQKV], f32, tag="psqkv")
                    for cc in range(CC):
                        if cc == CC // 2 and mid is not None:
                            mid()  # splice transposes mid-accumulation
                        nc.tensor.matmul(
                            ps, xt_all[:, cc, tt * 128:(tt + 1) * 128],
                            wqkv_sb[:, cc, :],
                            start=(cc == 0), stop=(cc == CC - 1))
                    return ps

                def emit_rope(tt, ps):
                    """rms-norm + rope; q/k hop PSUM->SBUF fp16 once (ACT),
                    then all elementwise work runs fp16.  Returns (qr, kr)."""
                    QK = (H_LOC + 1) * HD  # q heads + k
                    qk16 = p1t.tile([128, QK], f16, tag="qk16")
                    nc.scalar.activation(qk16, ps[:, 0:QK], AF.Copy)
                    # v copies: ACT from PSUM, then Pool SBUF->SBUF
                    nc.scalar.activation(v_aug[:, tt, 0:64],
                                         ps[:, QK:QK + HD], AF.Copy)
                    nc.gpsimd.tensor_copy(v_aug2[:, tt, 64:128],
                                          v_aug[:, tt, 0:64])

                    # sum of squares per head (Pool), 1/rms (ACT sqrt + DVE)
                    q5 = qk16.rearrange("p (h d) -> p h d", h=H_LOC + 1)
                    sq16 = p1t.tile([128, H_LOC + 1, HD], f16, tag="sq16")
                    nc.gpsimd.tensor_mul(sq16, q5, q5)
                    ssk = p1t.tile([128, H_LOC + 1, 1], f32, tag="ssk")
                    nc.vector.reduce_sum(ssk, sq16, axis=AX.X)
                    rinv = p1t.tile([128, H_LOC + 1], f32, tag="rinv")
                    nc.scalar.activation(
                        rinv, ssk.rearrange("p h o -> p (h o)"), AF.Sqrt,
                        bias=eps_t[:, 0:1], scale=1.0 / HD)
                    nc.vector.reciprocal(rinv, rinv)
                    r16 = p1t.tile([128, H_LOC + 1, 1], f16, tag="r16")
                    nc.vector.tensor_copy(
                        r16, rinv.rearrange("p (h o) -> p h o", o=1))

                    # rope(q) * rinv_q (fp16 all the way; rms-norm commutes
                    # with rope so the 1/rms multiply comes last)
                    q3 = q5[:, 0:H_LOC, :]
                    cq = ctabs["cosq"][:, tt, :].rearrange(
                        "p (o d) -> p o d", o=1)
                    sq = ctabs["sinq"][:, tt, :].rearrange(
                        "p (o d) -> p o d", o=1)
                    qr_f = p1t.tile([128, H_LOC, HD], f16, tag="qr_f")
                    nc.vector.tensor_mul(
                        qr_f, q3, cq.to_broadcast([128, H_LOC, HD]))
                    qrot = p1t.tile([128, H_LOC, HD], f16, tag="qrot")
                    nc.vector.tensor_mul(
                        qrot[:, :, 0:32], q3[:, :, 32:64],
                        sq[:, :, 0:32].to_broadcast([128, H_LOC, 32]))
                    nc.vector.tensor_mul(
                        qrot[:, :, 32:64], q3[:, :, 0:32],
                        sq[:, :, 32:64].to_broadcast([128, H_LOC, 32]))
                    nc.vector.tensor_add(qr_f, qr_f, qrot)
                    qr = p1t.tile([128, H_LOC, HD], f16, tag="qr")
                    nc.vector.tensor_mul(
                        qr, qr_f,
                        r16[:, 0:H_LOC, :].to_broadcast([128, H_LOC, HD]))

                    # rope(k) * rinv_k
                    k1 = q5[:, H_LOC, :]
                    kr_f = p1t.tile([128, HD], f16, tag="kr_f")
                    nc.vector.tensor_mul(kr_f, k1, ctabs["cosk"][:, tt, :])
                    krot = p1t.tile([128, HD], f16, tag="krot")
                    nc.vector.tensor_mul(
                        krot[:, 0:32], k1[:, 32:64],
                        ctabs["sink"][:, tt, 0:32])
                    nc.vector.tensor_mul(
                        krot[:, 32:64], k1[:, 0:32],
                        ctabs["sink"][:, tt, 32:64])
                    nc.vector.tensor_add(kr_f, kr_f, krot)
                    kr = p1t.tile([128, HD], f16, tag="kr")
                    nc.vector.tensor_mul(
                        kr, kr_f,
                        r16[:, H_LOC, :].to_broadcast([128, HD]))
                    return qr, kr

                def emit_transpose(tt, qr, kr):
                    psqt = ps1a.tile([64, 512], f16, tag="psqt")
                    for h in range(H_LOC):
                        nc.tensor.transpose(
                            psqt[:, h * 128:(h + 1) * 128], qr[:, h, :],
                            ident)
                    # one strided DVE copy: psqt [64,(4,128)] -> qT[:,h,ttb]
                    nc.vector.tensor_copy(
                        qT[:, :, tt * 128:(tt + 1) * 128],
                        psqt.rearrange("p (h t) -> p h t", h=H_LOC))
                    pskt = ps1a.tile([64, 128], f16, tag="pskt")
                    nc.tensor.transpose(pskt, kr, ident)
                    nc.vector.tensor_copy(
                        kT[:, tt * 128:(tt + 1) * 128], pskt)

                # software-pipeline: transposes(tt-1) are emitted after
                # proj(tt) so the PE never waits on the DVE rope
                # transposes run TWO t-tiles behind the projection: the
                # rms+rope chain spans ~4 engines and needs ~2 proj-tiles
                # of PE time to finish without stalling the transposes
                from collections import deque
                lag = deque()
                for tt in range(TT):
                    mid = None
                    if len(lag) >= 2:
                        mid = (lambda p=lag.popleft():
                               emit_transpose(*p))
                    ps = emit_proj(tt, mid)
                    lag.append((tt,) + emit_rope(tt, ps))
                while lag:
                    emit_transpose(*lag.popleft())

            # ---- phase 2+3: attention + out-projection ----
            # masked-tile ets live from the score burst until the PV drain:
            # pool must hold max(masked)+L+2 or the in-order PE queue
            # deadlocks waiting on a buffer freed only later in the queue
            n_mask_max = max(
                sum(1 for jb in range(JBS)
                    if status[ib][jb] not in (SKIP, KEEP))
                for ib in range(IBS))
            with (
                tc.tile_pool(name="p2w", bufs=1) as p2w,
                tc.tile_pool(name="p2e", bufs=n_mask_max + 6) as p2e,
                tc.tile_pool(name="p2o", bufs=3) as p2o,
                tc.tile_pool(name="p2den", bufs=2) as p2den,
                tc.tile_pool(name="ps2s", bufs=2, space="PSUM") as ps2s,
                tc.tile_pool(name="ps2c", bufs=2, space="PSUM") as ps2c,
                tc.tile_pool(name="ps2o", bufs=2, space="PSUM") as ps2o,
            ):
                wo_sb = [p2w.tile([128, D], f16, name=f"wo{p}")
                         for p in range(2)]
                for p in range(2):
                    nc.sync.dma_start(
                        out=wo_sb[p], in_=wo_d[p * 128:(p + 1) * 128, :]
                    )
                mpat_sb = p2w.tile([128, n_pat, 512], f16)
                nc.sync.dma_start(
                    out=mpat_sb, in_=mpat_d.rearrange("n p f -> p n f")
                )

                pend = []  # deferred pair-finalize closures (cross-block)

                def emit_attention(ib, chunk_cb=None):
                    """scores+exp+PV for the 4 heads of i-block ib.

                    Masked (diagonal/pattern) tiles get their score matmul
                    FIRST and their PV matmul LAST, so the exp+mask chain on
                    ACT/Pool has the whole block's worth of PE work to hide
                    behind; unmasked tiles run a lead-L software pipeline.

                    Each head pair's softmax normalize is fused: 1/den rows
                    (PSUM partitions 64/32) -> fp16 -> PE partition-broadcast
                    -> one DVE multiply writing ctx16 straight from psc.
                    chunk_cb(k), called once per head, splices a slice of the
                    previous block's out-projection into this block so its
                    DVE copies never form a head-of-line block."""
                    masked = [jb for jb in range(JBS)
                              if status[ib][jb] not in (SKIP, KEEP)]
                    keeps = [jb for jb in range(JBS)
                             if status[ib][jb] == KEEP]
                    pv_order = keeps + masked    # accumulation emission
                    # psc's first accumulation must cover the full width
                    if not keeps:
                        full = [jb for jb in pv_order
                                if _affine_c0(ib, jb) == 0]
                        first = full[0] if full else pv_order[0]
                        pv_order.remove(first)
                        pv_order.insert(0, first)
                    dens = p2den.tile([65, 2, 512], f32, tag="dens")
                    d16 = p2den.tile([65, 2, 512], f16, tag="d16")
                    L = 3
                    ibb = slice(ib * 512, (ib + 1) * 512)
                    pscs = {}

                    def finalize(pair):
                        """1/den broadcast + fused normalize for both heads
                        of `pair` (deferred so PE work covers the chain)."""
                        nc.vector.tensor_copy(d16[64:65, pair, :],
                                              dens[64:65, pair, :])
                        nc.vector.tensor_copy(d16[32:33, pair, :],
                                              dens[32:33, pair, :])
                        dbc = ps2o.tile([128, 512], f32, tag="pso")
                        nc.tensor.matmul(dbc, ones65[64:65, :],
                                         d16[64:65, pair, :],
                                         start=True, stop=False)
                        nc.tensor.matmul(dbc, ones65[32:33, :],
                                         d16[32:33, pair, :],
                                         start=False, stop=True)
                        # DVE may read only one PSUM input: hop dbc to SBUF
                        dbs = p2den.tile([128, 512], f32, tag="dbs")
                        nc.vector.tensor_copy(dbs, dbc)
                        pe, po = pscs.pop((pair, 0)), pscs.pop((pair, 1))
                        nc.vector.tensor_mul(
                            ctx16[pair][0:64, ibb], pe[0:64, :],
                            dbs[0:64, :])
                        nc.vector.tensor_mul(
                            ctx16[pair][64:128, ibb], po[64:128, :],
                            dbs[64:128, :])

                    for pair in range(2):
                        psc_e = ps2c.tile([128, 512], f32, tag="psc")
                        psc_o = ps2c.tile([128, 512], f32, tag="psc")
                        pscs[(pair, 0)], pscs[(pair, 1)] = psc_e, psc_o
                        ets = {}

                        def emit_score(jb):
                            """both heads of the pair share kT: two score
                            matmuls into a 2-bank PSUM tile, ONE exp and ONE
                            mask op over [128, 2, w]."""
                            st = status[ib][jb]
                            c0 = _affine_c0(ib, jb) if st == AFFINE else 0
                            pss = ps2s.tile([128, 2, 512], f32, tag="pss")
                            # one matmul per head: a single 2-bank-spanning
                            # matmul fails the ISA check on HW
                            for s in range(2):
                                nc.tensor.matmul(
                                    pss[:, s, c0:512],
                                    kT[:, jb * 128:(jb + 1) * 128],
                                    qT[:, 2 * pair + s,
                                       ib * 512 + c0:(ib + 1) * 512],
                                    start=True, stop=True)
                            et = p2e.tile([128, 2, 512], f16, tag="et")
                            nc.scalar.activation(
                                et[:, :, c0:512], pss[:, :, c0:512], AF.Exp,
                                scale=inv_sqrt_d)
                            if st == AFFINE:
                                nc.gpsimd.affine_select(
                                    out=et[:, :, c0:512],
                                    in_=et[:, :, c0:512],
                                    compare_op=mybir.AluOpType.is_ge,
                                    fill=0.0,
                                    base=ib * 512 - jb * 128 + c0,
                                    pattern=[[0, 2], [1, 512 - c0]],
                                    channel_multiplier=-1,
                                )
                            elif isinstance(st, tuple):
                                nc.vector.tensor_mul(
                                    et, et,
                                    mpat_sb[:, st[1], :]
                                    .rearrange("p (o f) -> p o f", o=1)
                                    .to_broadcast([128, 2, 512]))
                            ets[jb] = (c0, et)

                        def emit_pv(m):
                            jb = pv_order[m]
                            c0, et = ets.pop(jb)
                            nc.tensor.matmul(
                                psc_e[0:65, c0:512],
                                v_aug[:, jb, :], et[:, 0, c0:512],
                                start=(m == 0),
                                stop=(m == len(pv_order) - 1))
                            nc.tensor.matmul(
                                psc_o[:, c0:512],
                                v_aug2[:, jb, :], et[:, 1, c0:512],
                                start=(m == 0),
                                stop=(m == len(pv_order) - 1))

                        # masked tiles' scores first: while the PE stalls on
                        # the 2-deep pss pool, ACT front-loads the slow
                        # masked exps and the keeps then pipeline cleanly
                        # (interleaving measures worse); PVs drain
                        # keeps-first, masked-last with a lead-L pipeline
                        for jb in masked:
                            emit_score(jb)
                        while pend:  # prev pair's finalize, under PE cover
                            pend.pop()()
                        if chunk_cb is not None:
                            chunk_cb(2 * pair)
                        pv = 0
                        did_mid = False
                        for n, jb in enumerate(keeps):
                            emit_score(jb)
                            if n == len(keeps) // 2 and chunk_cb and \
                                    not did_mid:
                                chunk_cb(2 * pair + 1)
                                did_mid = True
                            if n >= L:
                                emit_pv(pv)
                                pv += 1
                        while pv < len(pv_order):
                            emit_pv(pv)
                            pv += 1
                        if chunk_cb is not None and not did_mid:
                            chunk_cb(2 * pair + 1)
                        # 1/den straight from PSUM (no partition shift)
                        nc.vector.reciprocal(
                            dens[64:65, pair, :], psc_e[64:65, :])
                        nc.vector.reciprocal(
                            dens[32:33, pair, :], psc_o[32:33, :])
                        pend.append(lambda p=pair: finalize(p))

                def emit_outproj_tt(tt, tail=False):
                    """one t-tile (128 rows) of the out-projection; the
                    four column-blocks stage into one fp16 tile so a
                    single DMA stores the whole row (DMA issues cost
                    ~500ns of sequencer time each - 16 stores, not 64)"""
                    ttb = slice(tt * 128, (tt + 1) * 128)
                    otw = p2o.tile([128, 4, 512], f16, tag="otw")
                    for cb in range(4):
                        cbb = slice(cb * 512, (cb + 1) * 512)
                        pso = ps2o.tile([128, 512], f32, tag="pso")
                        nc.tensor.matmul(pso, ctx16[0][:, ttb],
                                         wo_sb[0][:, cbb],
                                         start=True, stop=False)
                        nc.tensor.matmul(pso, ctx16[1][:, ttb],
                                         wo_sb[1][:, cbb],
                                         start=False, stop=True)
                        # tail: ACT is idle (no more exps), split the drain;
                        # earlier: ACT copies would queue behind the next
                        # block's exps and hold the pso bank
                        if tail and cb % 2 == 1:
                            nc.scalar.activation(otw[:, cb, :], pso, AF.Copy)
                        else:
                            nc.vector.tensor_copy(otw[:, cb, :], pso)
                    nc.sync.dma_start(
                        out=out_d[ttb, :],
                        in_=otw.rearrange("p a b -> p (a b)"))

                # attention(ib) runs one i-block ahead of the normalize +
                # out-projection so the PE never waits on the den bounce
                if _PARTS == "p1":
                    nc.sync.dma_start(out=out_d[0:64, 0:512],
                                      in_=kT[:, 0:512])
                else:
                    # ascending: block ib's out-projection (constant size)
                    # hides under attention(ib+1), which is always larger
                    order = list(range(IBS))
                    prev_ib = None
                    for ib in order:
                        cb = None
                        if prev_ib is not None and _PARTS == "full":
                            cb = (lambda k, base=prev_ib * 4:
                                  emit_outproj_tt(base + k))
                        emit_attention(ib, cb)
                        prev_ib = ib
                    while pend:
                        pend.pop()()
                    if _PARTS == "full":
                        for k in range(4):
                            emit_outproj_tt(prev_ib * 4 + k, tail=True)

    _split_multiwaits(nc)
    return nc


_CACHE = {}


def _get_program(mask_key, status, n_pat, reps=1):
    key = (mask_key, reps)
    if key not in _CACHE:
        _CACHE[key] = _build_program(status, n_pat, reps)
    return _CACHE[key]


def _prepare(x, mask, cos, sin, W_query, W_key, W_value, W_out,
             q_scale, k_scale, reps=1):
    """Host-side prep: fold scales into rope tables, shard weights,
    classify the mask.  Returns (nc, in_maps)."""
    cos = np.asarray(cos, dtype=np.float32)
    sin = np.asarray(sin, dtype=np.float32)
    W_query = np.asarray(W_query, dtype=np.float32)
    W_key = np.asarray(W_key, dtype=np.float32)
    W_value = np.asarray(W_value, dtype=np.float32)
    W_out = np.asarray(W_out, dtype=np.float32)
    q_scale = np.asarray(q_scale, dtype=np.float32)
    k_scale = np.asarray(k_scale, dtype=np.float32)
    mask = np.asarray(mask)

    xf = np.ascontiguousarray(
        np.asarray(x).reshape(T, D).astype(np.float16)
    )

    # rope = qn*cos' + shuffle32(qn)*sin' with the rotate-half signs and the
    # post-norm q/k scales folded into the tables:
    #   rope(s*qn) = qn*(s*cos) + shuffle32(qn)*(shuffle32(s)*sin+-)
    def tables(scale):
        perm = np.concatenate([scale[HD // 2:], scale[:HD // 2]])
        c = (cos * scale[None, :]).astype(np.float32)
        s = (sin * perm[None, :]).astype(np.float32)
        s[:, :HD // 2] *= -1.0
        return np.ascontiguousarray(c), np.ascontiguousarray(s)

    cq, sq_t = tables(q_scale)
    ck, sk_t = tables(k_scale)
    cq, sq_t = cq.astype(np.float16), sq_t.astype(np.float16)
    ck, sk_t = ck.astype(np.float16), sk_t.astype(np.float16)

    status, patterns = _classify_mask(mask)
    nc = _get_program(mask.tobytes(), status, patterns.shape[0], reps)

    patterns = np.ascontiguousarray(patterns.astype(np.float16))
    in_maps = []
    for c in range(N_CORES):
        qcols = slice(c * H_LOC * HD, (c + 1) * H_LOC * HD)
        kvcols = slice(c * HD, (c + 1) * HD)
        wqkv = np.concatenate(
            [W_query[:, qcols], W_key[:, kvcols], W_value[:, kvcols]],
            axis=1).astype(np.float16)
        in_maps.append({
            "x": xf,
            "wqkv": np.ascontiguousarray(wqkv),
            "wo": np.ascontiguousarray(
                W_out[qcols, :].astype(np.float16)),
            "cosq": cq, "sinq": sq_t, "cosk": ck, "sink": sk_t,
            "mpat": patterns,
        })
    return nc, in_maps


def kernel(x, mask, cos, sin, W_query, W_key, W_value, W_out,
           q_scale, k_scale):
    out_dtype = np.asarray(x).dtype
    nc, in_maps = _prepare(x, mask, cos, sin, W_query, W_key, W_value,
                           W_out, q_scale, k_scale)

    from concourse.bass_utils import run_bass_kernel_spmd

    res = run_bass_kernel_spmd(nc, in_maps, list(range(N_CORES)))
    acc = res.results[0]["out"].astype(np.float32)
    for c in range(1, N_CORES):
        acc = acc + res.results[c]["out"].astype(np.float32)
    return acc.reshape(1, T, D).astype(out_dtype)
